# revision 4
# baseline (speedup 1.0000x reference)
"""BitLinear FFN (BitNet b1.58) Trainium2 kernel, 8-core SPMD.

Strategy: data-parallel over tokens. Every core processes 1024 of the 8192
tokens end-to-end. Weight quantization (+ transpose to contraction-major
layout, + cast to bf16) is sharded 1/8 per core and shared via one AllGather.

Exactness: activation quantization produces integers in [-127, 127] and
weight quantization produces {-1, 0, 1}; both are exact in bf16, and the PE
accumulates in fp32, so all three matmuls are exact integer arithmetic.
Per-token/per-tensor dequant scales are applied on the fly:
  gate = gate_int * c_g          (c_g per token, before silu)
  c_u cancels inside the second act-quant, so `up` stays in integer form
  out  = down_int * F_t          (F_t per token, fused into PSUM evacuation)
"""

import numpy as np

import concourse.bacc as bacc
import concourse.bass as bass
import concourse.bass_isa as bass_isa
import concourse.mybir as mybir
import concourse.tile as tile
from concourse.masks import make_identity

P = 128
HID = 1024
INNER = 4096
N_CORES = 8
T_CORE = 1024          # tokens per core
TC = 256               # token chunk in the main loop
NCH = T_CORE // TC     # 4 chunks
MT = TC // P           # 2 token tiles per chunk
KI = HID // P          # 8 contraction tiles for gate/up
KOG = INNER // P       # 32 contraction tiles for down
OSH = INNER // N_CORES  # 512, o-shard per core
HSH = HID // P         # 8 h-subtiles in w_down shard [1024, 512]

MROUND = 12582912.0    # 1.5 * 2**23: (v + M) - M == round-half-even(v)
W_ELEMS = float(INNER * HID)

F32 = mybir.dt.float32
BF16 = mybir.dt.bfloat16

# element offsets inside the per-core staging buffer (bf16 elements)
WG_OFF = 0
WU_OFF = OSH * HID          # 524288
WD_OFF = 2 * OSH * HID      # 1048576
STAGE_ELEMS = 3 * OSH * HID  # 1572864

A = mybir.AluOpType
AF = mybir.ActivationFunctionType


def build_bass(sim_mode: bool = False):
    """Build the SPMD program. sim_mode replaces collectives with local
    stand-ins so the single-core cost-model simulator can run it."""
    nc = bacc.Bacc(
        "TRN2", target_bir_lowering=False, debug=False,
        num_devices=N_CORES,
    )
    groups = [list(range(N_CORES))]

    x_d = nc.dram_tensor("x_shard", [T_CORE, HID], F32, kind="ExternalInput")
    wg_d = nc.dram_tensor("wg_shard", [OSH, HID], F32, kind="ExternalInput")
    wu_d = nc.dram_tensor("wu_shard", [OSH, HID], F32, kind="ExternalInput")
    wd_d = nc.dram_tensor("wd_shard", [HID, OSH], F32, kind="ExternalInput")
    out_d = nc.dram_tensor("out_shard", [T_CORE, HID], F32, kind="ExternalOutput")

    wg_r = wg_d.ap().rearrange("(po p) i -> po p i", p=P)    # [4, 128, 1024]
    wu_r = wu_d.ap().rearrange("(po p) i -> po p i", p=P)
    wd_r = wd_d.ap().rearrange("(hs p) o -> hs p o", p=P)    # [8, 128, 512]
    x_r = x_d.ap().rearrange("(n p) i -> n p i", p=P)        # [8, 128, 1024]
    out_r = out_d.ap().rearrange("(n p) h -> n p h", p=P)

    with tile.TileContext(nc) as tc:
        with (
            tc.tile_pool(name="const", bufs=1) as constp,
            tc.tile_pool(name="big", bufs=1) as bigp,
            tc.tile_pool(name="stream", bufs=2) as streamp,
            tc.tile_pool(name="stg", bufs=1) as stgp,
            tc.tile_pool(name="ew", bufs=2) as ewp,
            tc.tile_pool(name="outp", bufs=2) as outpp,
            tc.tile_pool(name="tiny", bufs=2) as tinyp,
            tc.tile_pool(name="pg", bufs=4, space="PSUM") as pgp,
            tc.tile_pool(name="pd", bufs=2, space="PSUM") as pdp,
            tc.tile_pool(name="pt", bufs=2, space="PSUM") as ptp,
            tc.tile_pool(name="dram", bufs=1, space="DRAM") as dramp,
        ):
            ident = constp.tile([P, P], BF16)
            make_identity(nc, ident)
            ones_col = constp.tile([P, 1], F32)
            nc.gpsimd.memset(ones_col[:], 1.0)

            # ---------------- |w| partial sums over this core's shards -----
            sums_col = constp.tile([P, 4], F32)
            nc.gpsimd.memset(sums_col[:], 0.0)
            for j, (src, n_sub) in enumerate(
                ((wg_r, 4), (wu_r, 4), (wd_r, HSH))
            ):
                for po in range(n_sub):
                    wld = streamp.tile([P, HID], F32, tag="wld")
                    nc.sync.dma_start(out=wld[:, :src.shape[2]], in_=src[po])
                    part = tinyp.tile([P, 1], F32, tag="wabs")
                    nc.vector.tensor_reduce(
                        out=part[:], in_=wld[:, :src.shape[2]],
                        axis=mybir.AxisListType.X,
                        op=A.add, apply_absolute_value=True)
                    nc.vector.tensor_tensor(
                        out=sums_col[:, j:j + 1], in0=sums_col[:, j:j + 1],
                        in1=part[:], op=A.add)

            psums = ptp.tile([1, 4], F32, tag="pt")
            nc.tensor.matmul(psums[:], lhsT=ones_col[:], rhs=sums_col[:],
                             start=True, stop=True)
            sums_sb = tinyp.tile([1, 4], F32)
            nc.vector.tensor_copy(out=sums_sb[:], in_=psums[:])

            # tiny AllReduce of the three |w| sums
            sums_in = dramp.tile([1, 4], F32)
            sums_out = dramp.tile([1, 4], F32, addr_space="Shared")
            nc.sync.dma_start(out=sums_in[:], in_=sums_sb[:])
            if sim_mode:
                nc.sync.dma_start(out=sums_out[:], in_=sums_in[:])
            else:
                nc.gpsimd.collective_compute(
                    "AllReduce", A.add, replica_groups=groups,
                    ins=[sums_in[:]], outs=[sums_out[:]])
            sums_all = tinyp.tile([1, 4], F32)
            nc.sync.dma_start(out=sums_all[:], in_=sums_out[:])

            # clip-means (= 1/s_w) and s_w, broadcast to all partitions
            mcl = tinyp.tile([1, 4], F32)
            nc.vector.tensor_scalar(
                out=mcl[:], in0=sums_all[:], scalar1=1.0 / W_ELEMS,
                scalar2=1e-5, op0=A.mult, op1=A.max)
            sw = tinyp.tile([1, 4], F32)
            nc.vector.reciprocal(out=sw[:], in_=mcl[:])
            swb = constp.tile([P, 4], F32)
            nc.gpsimd.partition_broadcast(swb[:], sw[0:1, :])
            mclb = constp.tile([P, 4], F32)
            nc.gpsimd.partition_broadcast(mclb[:], mcl[0:1, :])
            # bc_ud = clip_mean_wu * clip_mean_wd / 127^2   (for F_t)
            bc_ud = constp.tile([P, 1], F32)
            nc.vector.tensor_tensor(
                out=bc_ud[:], in0=mclb[:, 1:2], in1=mclb[:, 2:3], op=A.mult)
            nc.vector.tensor_scalar_mul(bc_ud[:], bc_ud[:], 1.0 / (127.0 * 127.0))

            # ---------------- quantize + transpose weight shards -----------
            stage_d = dramp.tile([STAGE_ELEMS], BF16)
            ag_d = dramp.tile([N_CORES, STAGE_ELEMS], BF16, addr_space="Shared")

            def quant_tile(src_sb, w_idx, width):
                """bf16 tile <- clip(round(src * s_w), -1, 1); src clobbered."""
                sw_col = swb[:, w_idx:w_idx + 1]
                nc.vector.tensor_scalar(
                    out=src_sb, in0=src_sb, scalar1=sw_col, scalar2=MROUND,
                    op0=A.mult, op1=A.add)
                wq_b = streamp.tile([P, HID], BF16, tag="wqb")
                nc.vector.tensor_scalar(
                    out=wq_b[:, :width], in0=src_sb, scalar1=-MROUND,
                    scalar2=1.0, op0=A.add, op1=A.min)
                nc.vector.tensor_scalar_max(
                    wq_b[:, :width], wq_b[:, :width], -1.0)
                return wq_b

            # gate / up: stage layout [KI, 128 i, OSH o]
            for src, off, w_idx in ((wg_r, WG_OFF, 0), (wu_r, WU_OFF, 1)):
                stage_sb = stgp.tile([P, KI, OSH], BF16, tag="stg")
                for po in range(4):
                    wld = streamp.tile([P, HID], F32, tag="wld")
                    nc.sync.dma_start(out=wld[:], in_=src[po])
                    wq_b = quant_tile(wld[:], w_idx, HID)
                    for ki in range(KI):
                        pt_t = ptp.tile([P, P], BF16, tag="pt")
                        nc.tensor.transpose(
                            pt_t[:], wq_b[:, ki * P:(ki + 1) * P], ident[:])
                        nc.vector.tensor_copy(
                            out=stage_sb[:, ki, po * P:(po + 1) * P], in_=pt_t[:])
                nc.sync.dma_start(
                    out=stage_d[off:off + OSH * HID].rearrange(
                        "(ki i o) -> i ki o", ki=KI, i=P, o=OSH),
                    in_=stage_sb[:])

            # down: stage layout [4 ko, 128 o, HID h]
            stage_sb = stgp.tile([P, 4, HID], BF16, tag="stg")
            for hs in range(HSH):
                wld = streamp.tile([P, HID], F32, tag="wld")
                nc.sync.dma_start(out=wld[:, :OSH], in_=wd_r[hs])
                wq_b = quant_tile(wld[:, :OSH], 2, OSH)
                for ko in range(4):
                    pt_t = ptp.tile([P, P], BF16, tag="pt")
                    nc.tensor.transpose(
                        pt_t[:], wq_b[:, ko * P:(ko + 1) * P], ident[:])
                    nc.vector.tensor_copy(
                        out=stage_sb[:, ko, hs * P:(hs + 1) * P], in_=pt_t[:])
            nc.sync.dma_start(
                out=stage_d[WD_OFF:WD_OFF + OSH * HID].rearrange(
                    "(ko o h) -> o ko h", ko=4, o=P, h=HID),
                in_=stage_sb[:])

            # AllGather the quantized transposed shards
            if sim_mode:
                for c in range(N_CORES):
                    nc.sync.dma_start(out=ag_d[c, :], in_=stage_d[:])
            else:
                nc.gpsimd.collective_compute(
                    "AllGather", A.bypass, replica_groups=groups,
                    ins=[stage_d[:]], outs=[ag_d[:]])

            ag_wg = [
                ag_d[c, WG_OFF:WG_OFF + OSH * HID].rearrange(
                    "(ki i o) -> i ki o", ki=KI, i=P, o=OSH)
                for c in range(N_CORES)
            ]
            ag_wu = [
                ag_d[c, WU_OFF:WU_OFF + OSH * HID].rearrange(
                    "(ki i o) -> i ki o", ki=KI, i=P, o=OSH)
                for c in range(N_CORES)
            ]

            # ---------------- x shard: quantize + transpose -----------------
            xqT = bigp.tile([P, KI, T_CORE], BF16, tag="xqT")   # [i, t]
            absm_c = constp.tile([P, KI], F32)                  # clip(absmax_x)

            for ts in range(KI):  # 8 token tiles of 128
                x_sb = streamp.tile([P, HID], F32, tag="xld")
                nc.sync.dma_start(out=x_sb[:], in_=x_r[ts])
                am = tinyp.tile([P, 1], F32, tag="am")
                nc.vector.tensor_reduce(
                    out=am[:], in_=x_sb[:], axis=mybir.AxisListType.X,
                    op=A.max, apply_absolute_value=True)
                nc.vector.tensor_scalar_max(absm_c[:, ts:ts + 1], am[:], 1e-5)
                s1c = tinyp.tile([P, 1], F32, tag="s1c")
                nc.vector.reciprocal(out=s1c[:], in_=absm_c[:, ts:ts + 1])
                nc.vector.tensor_scalar_mul(s1c[:], s1c[:], 127.0)
                # xq = round(x * s1) in-place then cast
                nc.vector.tensor_scalar(
                    out=x_sb[:], in0=x_sb[:], scalar1=s1c[:, 0:1],
                    scalar2=MROUND, op0=A.mult, op1=A.add)
                xq_b = streamp.tile([P, HID], BF16, tag="wqb")
                nc.vector.tensor_scalar(
                    out=xq_b[:], in0=x_sb[:], scalar1=-MROUND, scalar2=None,
                    op0=A.add)
                for ki in range(KI):
                    pt_t = ptp.tile([P, P], BF16, tag="pt")
                    nc.tensor.transpose(
                        pt_t[:], xq_b[:, ki * P:(ki + 1) * P], ident[:])
                    nc.vector.tensor_copy(
                        out=xqT[:, ki, ts * P:(ts + 1) * P], in_=pt_t[:])

            # c_g: clip(absmax_x) * clip_mean_wg / 127, as [1, T] row
            cg_col = constp.tile([P, KI], F32)
            nc.vector.tensor_scalar(
                out=cg_col[:], in0=absm_c[:], scalar1=mclb[:, 0:1],
                scalar2=1.0 / 127.0, op0=A.mult, op1=A.mult)
            cg_row = constp.tile([1, T_CORE], F32)
            for ts in range(KI):
                nc.sync.dma_start(
                    out=cg_row[0:1, ts * P:(ts + 1) * P],
                    in_=cg_col[:, ts:ts + 1])

            # ---------------- cached transposed w_down ---------------------
            wdt4 = bigp.tile([P, N_CORES, 4, HID], BF16, tag="wdt")  # [o, c, ko, h]
            for c in range(N_CORES):
                nc.sync.dma_start(
                    out=wdt4[:, c],
                    in_=ag_d[c, WD_OFF:WD_OFF + OSH * HID].rearrange(
                        "(ko o h) -> o ko h", ko=4, o=P, h=HID))
            wdt = wdt4[:].rearrange("o c ko h -> o (c ko) h")

            # ---------------- main loop over token chunks -------------------
            for ch in range(NCH):
                tsl = slice(ch * TC, (ch + 1) * TC)
                cgb = ewp.tile([P, TC], F32, tag="cgb")
                nc.gpsimd.partition_broadcast(cgb[:], cg_row[0:1, tsl])

                prod = bigp.tile([P, KOG, TC], F32, tag="prod")
                for m in range(KOG):
                    c_src, po = divmod(m, 4)
                    osl = slice(po * P, (po + 1) * P)
                    wgt = streamp.tile([P, KI, P], BF16, tag="wgt")
                    nc.sync.dma_start(out=wgt[:], in_=ag_wg[c_src][:, :, osl])
                    wut = streamp.tile([P, KI, P], BF16, tag="wut")
                    nc.sync.dma_start(out=wut[:], in_=ag_wu[c_src][:, :, osl])

                    psg = pgp.tile([P, TC], F32, tag="pg")
                    for ki in range(KI):
                        nc.tensor.matmul(
                            psg[:], lhsT=wgt[:, ki], rhs=xqT[:, ki, tsl],
                            start=(ki == 0), stop=(ki == KI - 1))
                    psu = pgp.tile([P, TC], F32, tag="pg")
                    for ki in range(KI):
                        nc.tensor.matmul(
                            psu[:], lhsT=wut[:, ki], rhs=xqT[:, ki, tsl],
                            start=(ki == 0), stop=(ki == KI - 1))
                    # gate*c_g -> silu -> * up_int
                    gsc = ewp.tile([P, TC], F32, tag="gsc")
                    nc.vector.tensor_tensor(
                        out=gsc[:], in0=psg[:], in1=cgb[:], op=A.mult)
                    gsil = ewp.tile([P, TC], F32, tag="gsil")
                    nc.scalar.activation(gsil[:], gsc[:], AF.Silu)
                    nc.vector.tensor_tensor(
                        out=prod[:, m], in0=gsil[:], in1=psu[:], op=A.mult)

                # second act-quant: absmax over o (free sub-dim + partitions)
                maxr = ewp.tile([P, TC], F32, tag="maxr")
                nc.vector.tensor_reduce(
                    out=maxr[:], in_=prod[:].rearrange("p k t -> p t k"),
                    axis=mybir.AxisListType.X, op=A.max,
                    apply_absolute_value=True)
                maxg = ewp.tile([P, TC], F32, tag="maxg")
                nc.gpsimd.partition_all_reduce(
                    maxg[:], maxr[:], channels=P,
                    reduce_op=bass_isa.ReduceOp.max)
                nc.vector.tensor_scalar_max(maxg[:], maxg[:], 1e-5)
                s2b = ewp.tile([P, TC], F32, tag="s2b")
                nc.vector.reciprocal(out=s2b[:], in_=maxg[:])
                nc.vector.tensor_scalar_mul(s2b[:], s2b[:], 127.0)

                prodq = bigp.tile([P, KOG, TC], BF16, tag="prodq")
                nc.vector.tensor_tensor(
                    out=prod[:], in0=prod[:],
                    in1=s2b[:, None, :].to_broadcast((P, KOG, TC)),
                    op=A.mult)
                nc.vector.tensor_scalar(
                    out=prodq[:], in0=prod[:], scalar1=MROUND, scalar2=-MROUND,
                    op0=A.add, op1=A.add)

                # F_t column form for this chunk
                fcol = tinyp.tile([P, MT], F32, tag="fcol")
                for mt in range(MT):
                    nc.sync.dma_start(
                        out=fcol[:, mt:mt + 1],
                        in_=maxg[0:1, mt * P:(mt + 1) * P])
                nc.vector.tensor_tensor(
                    out=fcol[:], in0=fcol[:],
                    in1=absm_c[:, ch * MT:(ch + 1) * MT], op=A.mult)
                nc.vector.tensor_scalar_mul(fcol[:], fcol[:], bc_ud[:, 0:1])

                # down projection
                for mt in range(MT):
                    t0 = mt * P
                    for hh in range(2):
                        hsl = slice(hh * 512, (hh + 1) * 512)
                        psd = pdp.tile([P, 512], F32, tag="pd")
                        for kog in range(KOG):
                            nc.tensor.matmul(
                                psd[:], lhsT=prodq[:, kog, t0:t0 + P],
                                rhs=wdt[:, kog, hsl],
                                start=(kog == 0), stop=(kog == KOG - 1))
                        osb = outpp.tile([P, 512], F32, tag="osb")
                        nc.scalar.activation(
                            osb[:], psd[:], AF.Copy, scale=fcol[:, mt:mt + 1])
                        nc.sync.dma_start(
                            out=out_r[ch * MT + mt][:, hsl], in_=osb[:])

    nc.compile()
    return nc


_NC_CACHE = {}


def _get_nc():
    if "nc" not in _NC_CACHE:
        _NC_CACHE["nc"] = build_bass(sim_mode=False)
    return _NC_CACHE["nc"]


def make_in_maps(x, w_gate, w_up, w_down):
    x2 = np.ascontiguousarray(
        np.asarray(x, dtype=np.float32).reshape(N_CORES * T_CORE, HID))
    wg = np.asarray(w_gate, dtype=np.float32)
    wu = np.asarray(w_up, dtype=np.float32)
    wd = np.asarray(w_down, dtype=np.float32)
    in_maps = []
    for c in range(N_CORES):
        in_maps.append({
            "x_shard": np.ascontiguousarray(x2[c * T_CORE:(c + 1) * T_CORE]),
            "wg_shard": np.ascontiguousarray(wg[c * OSH:(c + 1) * OSH]),
            "wu_shard": np.ascontiguousarray(wu[c * OSH:(c + 1) * OSH]),
            "wd_shard": np.ascontiguousarray(wd[:, c * OSH:(c + 1) * OSH]),
        })
    return in_maps


def assemble_output(results):
    parts = [results[c]["out_shard"] for c in range(N_CORES)]
    return np.concatenate(parts, axis=0).reshape(4, 2048, HID)


def kernel(x, w_gate, w_up, w_down):
    from concourse.bass_utils import run_bass_kernel_spmd
    nc = _get_nc()
    in_maps = make_in_maps(x, w_gate, w_up, w_down)
    res = run_bass_kernel_spmd(nc, in_maps, list(range(N_CORES)), trace=False)
    return assemble_output(res.results)
